# revision 37
# baseline (speedup 1.0000x reference)
"""Chamfer-style loss kernel for Trainium2 (8 NeuronCores, SPMD).

Problem: y_pred [8192,2], y_true [8192,2] (fp32).
  d[n,m] = ||p_n - t_m||;  loss = (sum_n min_m d + sum_m min_n d) / 8192

Strategy per core k (shard y_pred rows, 1024 per core):
  - fp16 hi/lo-split augmented K=10 matmul on PE computes the NEGATED
    squared-distance tile S'[n,m] = -(|p_n|^2 + |t_m|^2 - 2 p.t) in PSUM
    at 1 cycle/column (4x the fp32 rate) with ~1e-6 absolute error:
      p = p_hi + p_lo (fp16 split), same for t and the squared norms;
      all lhs rows negated so min-of-distances becomes max-of-S'.
    K=10 uses 10 PE rows; 4 matmuls run concurrently on the four PE row
    quadrants via tile_position (lhs/rhs replicated at offsets 0/32/64/96).
  - ACT copies PSUM->SBUF as bf16 (max chains run at DVE 2x mode in bf16).
  - Reduction work is split across DVE and GpSimd: DVE does the two big
    row-max TTs and the col-tree first level; GpSimd does the col-tree
    tail (t2, t3) plus partition_all_reduce(max), which collapses all
    128 partitions to the per-column max in one op - no transposes.
  - sqrt commutes with min/max, so sqrt only on the 16K final extrema.
  - AllReduce(max) #1 over [1,4096] fp32 (negated col mins of the first
    4096 columns) launches mid-loop and overlaps the rest of the loop.
  - AllReduce(max) #2 over [1,4104] fp32: cols 4096..8191 plus 8 one-hot
    slots (-BIG elsewhere) carrying each core's row-min sqrt-sum,
    gathered via the max op itself.
  - Every core then computes the identical final scalar; host takes core 0.
"""

import sys

if "/opt/trn_rl_repo" not in sys.path:
    sys.path.insert(0, "/opt/trn_rl_repo")

import numpy as np

import concourse.bass as bass
import concourse.bacc as bacc
import concourse.bass_isa as bass_isa
import concourse.tile as tile
from concourse import mybir
from concourse.bass_utils import run_bass_kernel_spmd

F32 = mybir.dt.float32
F16 = mybir.dt.float16
BF16 = mybir.dt.bfloat16
MAX = mybir.AluOpType.max
MULT = mybir.AluOpType.mult
ADD = mybir.AluOpType.add
X = mybir.AxisListType.X

N_CORES = 8
N = 8192          # y_pred points
M = 8192          # y_true points
N_LOC = N // N_CORES   # 1024 rows per core
CHUNK = 512
N_PAIR = 8             # pairs of 512-column chunks (1024 cols each)
K = 10                 # augmented contraction depth (fp16 hi/lo split)
BIG = 3.0e38

TRACE = False          # set True by test harness to capture HW profile
LAST_RESULTS = None    # BassKernelResults of the most recent run

_CACHE = {}


def _build_program():
    nc = bacc.Bacc(
        "TRN2",
        target_bir_lowering=False,
        debug=False,
        num_devices=N_CORES,
    )

    lhs_d = nc.dram_tensor("lhs", [4 * K, N_LOC], F16, kind="ExternalInput")
    rhs_d = nc.dram_tensor("rhs", [4 * K, M], F16, kind="ExternalInput")
    out_d = nc.dram_tensor("out", [1, 1], F32, kind="ExternalOutput")

    with tile.TileContext(nc) as tc:
        with (
            tc.tile_pool(name="const", bufs=1) as const_pool,
            tc.tile_pool(name="acc", bufs=1) as acc_pool,
            tc.tile_pool(name="chunk", bufs=3) as chunk_pool,
            tc.tile_pool(name="tree", bufs=2) as tree_pool,
            tc.tile_pool(name="fin", bufs=1) as fin_pool,
            tc.tile_pool(name="mm", bufs=2, space="PSUM") as mm_pool,
            tc.tile_pool(name="dram", bufs=1, space="DRAM") as dram_pool,
        ):
            # ---- constants / inputs to SBUF ----
            # lhs/rhs live at partition offsets 0/32/64/96 so four K=10
            # matmuls can run concurrently on the four PE row quadrants.
            lhs_sb = const_pool.tile([128, N_LOC], F16, padded_shape=[128, N_LOC])
            rhs_sb = const_pool.tile([128, M], F16, padded_shape=[128, M])
            ones_sb = const_pool.tile([128, 1], F32)

            # lhs first (the weights gate the very first LDWEIGHTS), then
            # rhs with the first pair's columns in small leading pieces so
            # the first matmuls unblock on a ~1us transfer, not a ~4us one.
            # gpsimd's queue is kept clear for partition_all_reduce work.
            engs = [nc.sync, nc.scalar, nc.gpsimd]
            for r in range(4):
                engs[r % 3].dma_start(
                    lhs_sb[32 * r:32 * r + K, :], lhs_d.ap()[K * r:K * r + K, :]
                )
            for piece, (lo, hi) in enumerate([(0, 1024), (1024, 4096), (4096, 8192)]):
                for r in range(4):
                    engs[(piece * 4 + r + 1) % 3].dma_start(
                        rhs_sb[32 * r:32 * r + K, lo:hi],
                        rhs_d.ap()[K * r:K * r + K, lo:hi],
                    )
            nc.vector.memset(ones_sb[:, :], 1.0)

            # ---- persistent accumulators ----
            # row-max candidates, ping-pong: [128, (g:2, r:4, f:512)]
            rowacc_a = acc_pool.tile([128, 4096], BF16)
            rowacc_b = acc_pool.tile([128, 4096], BF16)
            rowaccs = [rowacc_a, rowacc_b]

            # [128, 32/33] payload shapes parallelize the collective across
            # partitions (a [1, N] payload serializes on one partition row).
            # AllGather + a local 8-way fold is ~2x cheaper than AllReduce.
            ag1_in = dram_pool.tile([128, 32], BF16)
            ag1_out = dram_pool.tile([1024, 32], BF16, addr_space="Shared")
            ag2_in = dram_pool.tile([128, 33], BF16)
            ag2_out = dram_pool.tile([1024, 33], BF16, addr_space="Shared")

            # ---- main loop over pairs of 512-column chunks ----
            for p in range(N_PAIR):
                if p == 4:
                    # cols 0..4095 complete: launch AllGather #1 now so it
                    # overlaps the rest of the loop (trigger emitted before
                    # pair 4's PAR so the gpsimd queue wait overlaps DVE)
                    nc.gpsimd.collective_compute(
                        "AllGather",
                        mybir.AluOpType.bypass,
                        replica_groups=[list(range(N_CORES))],
                        ins=[ag1_in[:, :].opt()],
                        outs=[ag1_out[:, :].opt()],
                    )
                pair_sb = chunk_pool.tile(
                    [128, 8192], BF16, name="pair_sb", tag="chunk"
                )
                # PE: S' tiles, 4 row-blocks at a time on the PE quadrants.
                # h = (g, cj): pair_sb layout [q, g:2, cj:2, r:4, f:512]
                for h in range(4):
                    g = h // 2
                    c = 2 * p + h % 2
                    mm_ps = mm_pool.tile(
                        [128, 4 * CHUNK], F32, name="mm_ps", tag="mm"
                    )
                    for r in range(4):
                        i = 4 * g + r
                        nc.tensor.matmul(
                            mm_ps[:, r * CHUNK:(r + 1) * CHUNK],
                            lhs_sb[32 * r:32 * r + K, i * 128:(i + 1) * 128],
                            rhs_sb[32 * r:32 * r + K, c * CHUNK:(c + 1) * CHUNK],
                            start=True,
                            stop=True,
                            tile_position=(32 * r, 0),
                        )
                    # ACT: PSUM fp32 -> SBUF bf16
                    nc.scalar.copy(
                        pair_sb[:, h * 2048:(h + 1) * 2048], mm_ps[:, :]
                    )

                pr = pair_sb.rearrange("q (g cj f) -> q g cj f", g=2, cj=2)
                if p == 0:
                    # row chain first and split per-g: the g=0 half starts
                    # after only two ACT copies (shrinks the pipeline head)
                    for g in range(2):
                        nc.vector.tensor_tensor(
                            rowacc_b[:, 2048 * g:2048 * (g + 1)],
                            pr[:, g, 0, :],
                            pr[:, g, 1, :],
                            MAX,
                        )

                # col tree (it feeds the PAR + collective chain):
                # contract g, then r-halves, on DVE.
                # t3[q, cj*512+f] = column m = 1024p + cj*512+f
                t1 = tree_pool.tile([128, 4096], BF16, name="t1", tag="t1")
                nc.vector.tensor_tensor(
                    t1[:, :], pair_sb[:, 0:4096], pair_sb[:, 4096:8192], MAX
                )
                t2 = tree_pool.tile([128, 2048], BF16, name="t2", tag="t2")
                t1v = t1.rearrange("q (cj r f) -> q cj r f", cj=2, r=4)
                t2v = t2.rearrange("q (cj r f) -> q cj r f", cj=2, r=2)
                nc.vector.tensor_tensor(
                    t2v[:, :, :, :], t1v[:, :, 0:2, :], t1v[:, :, 2:4, :], MAX
                )
                t3 = tree_pool.tile([128, 1024], BF16, name="t3", tag="t3")
                nc.vector.tensor_tensor(
                    t3.rearrange("q (cj f) -> q cj f", cj=2),
                    t2v[:, :, 0, :],
                    t2v[:, :, 1, :],
                    MAX,
                )
                # GpSimd partition fold: per-column max over all 128
                # partitions (result broadcast to every partition); dtype
                # matches the collective payload it feeds
                parx = tree_pool.tile([128, 1024], BF16, name="parx", tag="parx")
                nc.gpsimd.partition_all_reduce(
                    parx[:, :], t3[:, :], channels=128,
                    reduce_op=bass_isa.ReduceOp.max,
                )
                # stream partition 0 out to the collective payload:
                # pair p covers cols [8j+k] -> (partition j, col 8p'+k)
                if p < 4:
                    nc.sync.dma_start(
                        ag1_in[:, 8 * p:8 * (p + 1)], parx[0:1, :]
                    )
                else:
                    nc.sync.dma_start(
                        ag2_in[:, 8 * (p - 4):8 * (p - 3)], parx[0:1, :]
                    )

                # DVE row chain: max across the two chunks (cj), then into
                # rowacc.  view [q, g, cj, f'] with f' = r*512+f = 2048
                if p > 0:
                    trow = tree_pool.tile(
                        [128, 4096], BF16, name="trow", tag="trow"
                    )
                    nc.vector.tensor_tensor(
                        trow.rearrange("q (g f) -> q g f", g=2),
                        pr[:, :, 0, :],
                        pr[:, :, 1, :],
                        MAX,
                    )
                    src = rowaccs[p % 2]
                    dst = rowaccs[(p + 1) % 2]
                    nc.vector.tensor_tensor(dst[:, :], src[:, :], trow[:, :], MAX)

            # ---- local row-min finalization (negated: max chains) ----
            rowacc = rowaccs[N_PAIR % 2]
            rview = rowacc.rearrange("q (i f) -> q i f", i=8)
            rt1 = fin_pool.tile([128, 2048], BF16)
            rt2 = fin_pool.tile([128, 1024], BF16)
            r1v = rt1.rearrange("q (i f) -> q i f", i=8)
            r2v = rt2.rearrange("q (i f) -> q i f", i=8)
            nc.vector.tensor_tensor(
                r1v[:, :, :], rview[:, :, 0:256], rview[:, :, 256:512], MAX
            )
            nc.vector.tensor_tensor(
                r2v[:, :, :], r1v[:, :, 0:128], r1v[:, :, 128:256], MAX
            )
            # two more cheap 2x TT levels before the (1x-rate) reduce
            rt3 = fin_pool.tile([128, 512], BF16)
            rt4 = fin_pool.tile([128, 256], BF16)
            r3v = rt3.rearrange("q (i f) -> q i f", i=8)
            r4v = rt4.rearrange("q (i f) -> q i f", i=8)
            nc.vector.tensor_tensor(
                r3v[:, :, :], r2v[:, :, 0:64], r2v[:, :, 64:128], MAX
            )
            nc.vector.tensor_tensor(
                r4v[:, :, :], r3v[:, :, 0:32], r3v[:, :, 32:64], MAX
            )
            rowmax8 = fin_pool.tile([128, 8], F32)
            nc.vector.tensor_reduce(rowmax8[:, :], r4v, axis=X, op=MAX)

            # negate back to min-sq and clamp cancellation noise at 0:
            # rowmin8 = max(-rowmax8, 0), then sqrt and sum over free dim
            rowmin8 = fin_pool.tile([128, 8], F32)
            nc.vector.tensor_scalar(
                rowmin8[:, :], rowmax8[:, :], -1.0, 0.0, MULT, MAX
            )
            rowd = fin_pool.tile([128, 8], F32)
            rowpart = fin_pool.tile([128, 1], F32)
            nc.scalar.activation(
                rowd[:, :], rowmin8[:, :],
                mybir.ActivationFunctionType.Sqrt,
                accum_out=rowpart[:, :],
            )
            # partition sum -> scalar
            ps_row = mm_pool.tile([128, 4 * CHUNK], F32, name="ps_row", tag="mm")
            nc.tensor.matmul(
                ps_row[0:1, 0:1], ones_sb[:, :], rowpart[:, :],
                start=True, stop=True,
            )
            rowsum_sb = fin_pool.tile([1, 1], BF16)
            nc.scalar.copy(rowsum_sb[:, :], ps_row[0:1, 0:1])
            # this core's row-min sqrt-sum rides along in its own gathered
            # block at (row 0, col 32); other rows of col 32 are never read
            nc.scalar.dma_start(ag2_in[0:1, 32:33], rowsum_sb[:, :])

            # ---- AllGather #2 over [128, 33] fp32 ----
            nc.gpsimd.collective_compute(
                "AllGather",
                mybir.AluOpType.bypass,
                replica_groups=[list(range(N_CORES))],
                ins=[ag2_in[:, :].opt()],
                outs=[ag2_out[:, :].opt()],
            )

            # AG#1 consumption overlaps the AllGather #2 collective:
            # gather the 8 blocks (one strided DMA) and fold with a max tree
            cA8 = fin_pool.tile([128, 8 * 32], BF16)
            cAv = cA8.rearrange("q (j k) -> q j k", j=8)
            nc.scalar.dma_start(
                cAv[:, :, :],
                ag1_out[:, :].rearrange("(j q) k -> q j k", j=8),
            )
            mA1 = fin_pool.tile([128, 4 * 32], BF16)
            mA2 = fin_pool.tile([128, 2 * 32], BF16)
            cmaxA = fin_pool.tile([128, 32], BF16)
            mA1v = mA1.rearrange("q (j k) -> q j k", j=4)
            mA2v = mA2.rearrange("q (j k) -> q j k", j=2)
            nc.vector.tensor_tensor(
                mA1v[:, :, :], cAv[:, 0:4, :], cAv[:, 4:8, :], MAX
            )
            nc.vector.tensor_tensor(
                mA2v[:, :, :], mA1v[:, 0:2, :], mA1v[:, 2:4, :], MAX
            )
            nc.vector.tensor_tensor(
                cmaxA.rearrange("q (j k) -> q j k", j=1),
                mA2v[:, 0:1, :],
                mA2v[:, 1:2, :],
                MAX,
            )
            cminA = fin_pool.tile([128, 32], F32)
            nc.vector.tensor_scalar(
                cminA[:, :], cmaxA[:, :], -1.0, 0.0, MULT, MAX
            )
            cdA = fin_pool.tile([128, 32], F32)
            colpartA = fin_pool.tile([128, 1], F32)
            nc.scalar.activation(
                cdA[:, :], cminA[:, :],
                mybir.ActivationFunctionType.Sqrt,
                accum_out=colpartA[:, :],
            )

            # ---- global finalization (identical on every core) ----
            cB8 = fin_pool.tile([128, 8 * 33], BF16)
            cBv = cB8.rearrange("q (j k) -> q j k", j=8)
            nc.sync.dma_start(
                cBv[:, :, :],
                ag2_out[:, :].rearrange("(j q) k -> q j k", j=8),
            )
            mB1 = fin_pool.tile([128, 4 * 32], BF16)
            mB2 = fin_pool.tile([128, 2 * 32], BF16)
            cmaxB = fin_pool.tile([128, 32], BF16)
            mB1v = mB1.rearrange("q (j k) -> q j k", j=4)
            mB2v = mB2.rearrange("q (j k) -> q j k", j=2)
            nc.vector.tensor_tensor(
                mB1v[:, :, :], cBv[:, 0:4, 0:32], cBv[:, 4:8, 0:32], MAX
            )
            nc.vector.tensor_tensor(
                mB2v[:, :, :], mB1v[:, 0:2, :], mB1v[:, 2:4, :], MAX
            )
            nc.vector.tensor_tensor(
                cmaxB.rearrange("q (j k) -> q j k", j=1),
                mB2v[:, 0:1, :],
                mB2v[:, 1:2, :],
                MAX,
            )
            cminB = fin_pool.tile([128, 32], F32)
            nc.vector.tensor_scalar(
                cminB[:, :], cmaxB[:, :], -1.0, 0.0, MULT, MAX
            )
            cdB = fin_pool.tile([128, 32], F32)
            colpartB = fin_pool.tile([128, 1], F32)
            nc.scalar.activation(
                cdB[:, :], cminB[:, :],
                mybir.ActivationFunctionType.Sqrt,
                accum_out=colpartB[:, :],
            )
            colpart = fin_pool.tile([128, 1], F32)
            nc.vector.tensor_tensor(
                colpart[:, :], colpartA[:, :], colpartB[:, :], ADD
            )
            ps_col = mm_pool.tile([128, 4 * CHUNK], F32, name="ps_col", tag="mm")
            nc.tensor.matmul(
                ps_col[0:1, 0:1], ones_sb[:, :], colpart[:, :],
                start=True, stop=True,
            )
            colsum_sb = fin_pool.tile([1, 1], F32)
            nc.scalar.copy(colsum_sb[:, :], ps_col[0:1, 0:1])

            # all 8 cores' rowsums sit at (row 0, col 33j+32) of cB8
            rtot = fin_pool.tile([1, 1], F32)
            nc.vector.tensor_reduce(
                rtot[:, :], cBv[0:1, :, 32], axis=X, op=ADD
            )

            fin = fin_pool.tile([1, 1], F32)
            nc.vector.tensor_tensor(fin[:, :], colsum_sb[:, :], rtot[:, :], ADD)
            out_sb = fin_pool.tile([1, 1], F32)
            nc.scalar.mul(out_sb[:, :], fin[:, :], 1.0 / M)
            nc.sync.dma_start(out_d.ap(), out_sb[:, :])

    nc.compile()
    return nc


def _prep_inputs(y_pred, y_true):
    p = np.ascontiguousarray(np.asarray(y_pred, dtype=np.float32).reshape(-1, 2))
    t = np.ascontiguousarray(np.asarray(y_true, dtype=np.float32).reshape(-1, 2))
    assert p.shape == (N, 2) and t.shape == (M, 2)

    def split16(x):
        hi = x.astype(np.float16)
        lo = (x.astype(np.float64) - hi.astype(np.float64)).astype(np.float16)
        return hi, lo

    th, tl = split16(t)
    t2 = (t.astype(np.float64) ** 2).sum(1)
    t2h, t2l = split16(t2)
    ones_m = np.ones(M, np.float16)
    rhs10 = np.stack([
        th[:, 0], th[:, 1], th[:, 0], th[:, 1], t2h,
        ones_m, ones_m, tl[:, 0], tl[:, 1], t2l,
    ])
    rhs = np.ascontiguousarray(np.tile(rhs10, (4, 1)))

    in_maps = []
    for k in range(N_CORES):
        pk = p[k * N_LOC:(k + 1) * N_LOC]
        ph, pl = split16(pk)
        p2 = (pk.astype(np.float64) ** 2).sum(1)
        p2h, p2l = split16(p2)
        ones_n = np.ones(N_LOC, np.float16)
        # negated lhs: the PE emits S' = -(|p|^2 + |t|^2 - 2 p.t)
        lhs10 = np.stack([
            2.0 * ph[:, 0], 2.0 * ph[:, 1],
            2.0 * pl[:, 0], 2.0 * pl[:, 1],
            -ones_n, -p2h, -p2l,
            2.0 * ph[:, 0], 2.0 * ph[:, 1], -ones_n,
        ])
        lhs = np.ascontiguousarray(np.tile(lhs10, (4, 1)))
        in_maps.append({"lhs": lhs, "rhs": rhs})
    return in_maps


def kernel(y_pred, y_true):
    global LAST_RESULTS
    if "nc" not in _CACHE:
        _CACHE["nc"] = _build_program()
    nc = _CACHE["nc"]
    in_maps = _prep_inputs(y_pred, y_true)
    res = run_bass_kernel_spmd(
        nc,
        in_maps,
        core_ids=list(range(N_CORES)),
        trace=TRACE,
    )
    LAST_RESULTS = res
    return np.asarray(res.results[0]["out"], dtype=np.float32).reshape(())[()]


# revision 38
# speedup vs baseline: 1.7384x; 1.7384x over previous
"""Chamfer-style loss kernel for Trainium2 (8 NeuronCores, SPMD).

Problem: y_pred [8192,2], y_true [8192,2] (fp32).
  d[n,m] = ||p_n - t_m||;  loss = (sum_n min_m d + sum_m min_n d) / 8192

Strategy per core k (shard y_pred rows, 1024 per core):
  - fp16 hi/lo-split augmented K=10 matmul on PE computes the NEGATED
    squared-distance tile S'[n,m] = -(|p_n|^2 + |t_m|^2 - 2 p.t) in PSUM
    at 1 cycle/column (4x the fp32 rate) with ~1e-6 absolute error:
      p = p_hi + p_lo (fp16 split), same for t and the squared norms;
      all lhs rows negated so min-of-distances becomes max-of-S'.
    K=10 uses 10 PE rows; 4 matmuls run concurrently on the four PE row
    quadrants via tile_position (lhs/rhs replicated at offsets 0/32/64/96).
  - ACT copies PSUM->SBUF as bf16 (max chains run at DVE 2x mode in bf16).
  - Reduction work is split across DVE and GpSimd: DVE does the two big
    row-max TTs and the col-tree first level; GpSimd does the col-tree
    tail (t2, t3) plus partition_all_reduce(max), which collapses all
    128 partitions to the per-column max in one op - no transposes.
  - sqrt commutes with min/max, so sqrt only on the 16K final extrema.
  - AllReduce(max) #1 over [1,4096] fp32 (negated col mins of the first
    4096 columns) launches mid-loop and overlaps the rest of the loop.
  - AllReduce(max) #2 over [1,4104] fp32: cols 4096..8191 plus 8 one-hot
    slots (-BIG elsewhere) carrying each core's row-min sqrt-sum,
    gathered via the max op itself.
  - Every core then computes the identical final scalar; host takes core 0.
"""

import sys

if "/opt/trn_rl_repo" not in sys.path:
    sys.path.insert(0, "/opt/trn_rl_repo")

import numpy as np

import concourse.bass as bass
import concourse.bacc as bacc
import concourse.bass_isa as bass_isa
import concourse.tile as tile
from concourse import mybir
from concourse.bass_utils import run_bass_kernel_spmd

F32 = mybir.dt.float32
F16 = mybir.dt.float16
BF16 = mybir.dt.bfloat16
MAX = mybir.AluOpType.max
MULT = mybir.AluOpType.mult
ADD = mybir.AluOpType.add
X = mybir.AxisListType.X

N_CORES = 8
N = 8192          # y_pred points
M = 8192          # y_true points
N_LOC = N // N_CORES   # 1024 rows per core
CHUNK = 512
N_PAIR = 8             # pairs of 512-column chunks (1024 cols each)
K = 10                 # augmented contraction depth (fp16 hi/lo split)
BIG = 3.0e38

TRACE = False          # set True by test harness to capture HW profile
LAST_RESULTS = None    # BassKernelResults of the most recent run

_CACHE = {}


def _build_program():
    nc = bacc.Bacc(
        "TRN2",
        target_bir_lowering=False,
        debug=False,
        num_devices=N_CORES,
    )

    lhs_d = nc.dram_tensor("lhs", [4 * K, N_LOC], F16, kind="ExternalInput")
    rhs_d = nc.dram_tensor("rhs", [4 * K, M], F16, kind="ExternalInput")
    out_d = nc.dram_tensor("out", [1, 1], F32, kind="ExternalOutput")

    with tile.TileContext(nc) as tc:
        with (
            tc.tile_pool(name="const", bufs=1) as const_pool,
            tc.tile_pool(name="acc", bufs=1) as acc_pool,
            tc.tile_pool(name="chunk", bufs=3) as chunk_pool,
            tc.tile_pool(name="tree", bufs=2) as tree_pool,
            tc.tile_pool(name="fin", bufs=1) as fin_pool,
            tc.tile_pool(name="mm", bufs=2, space="PSUM") as mm_pool,
            tc.tile_pool(name="dram", bufs=1, space="DRAM") as dram_pool,
        ):
            # ---- constants / inputs to SBUF ----
            # lhs/rhs live at partition offsets 0/32/64/96 so four K=10
            # matmuls can run concurrently on the four PE row quadrants.
            lhs_sb = const_pool.tile([128, N_LOC], F16, padded_shape=[128, N_LOC])
            rhs_sb = const_pool.tile([128, M], F16, padded_shape=[128, M])
            ones_sb = const_pool.tile([128, 1], F32)

            # lhs first (the weights gate the very first LDWEIGHTS), then
            # rhs with the first pair's columns in small leading pieces so
            # the first matmuls unblock on a ~1us transfer, not a ~4us one.
            # gpsimd's queue is kept clear for partition_all_reduce work.
            engs = [nc.sync, nc.scalar, nc.gpsimd]
            for r in range(4):
                engs[r % 3].dma_start(
                    lhs_sb[32 * r:32 * r + K, :], lhs_d.ap()[K * r:K * r + K, :]
                )
            for piece, (lo, hi) in enumerate([(0, 1024), (1024, 4096), (4096, 8192)]):
                for r in range(4):
                    engs[(piece * 4 + r + 1) % 3].dma_start(
                        rhs_sb[32 * r:32 * r + K, lo:hi],
                        rhs_d.ap()[K * r:K * r + K, lo:hi],
                    )
            nc.vector.memset(ones_sb[:, :], 1.0)

            # ---- persistent accumulators ----
            # row-max candidates, ping-pong: [128, (g:2, r:4, f:512)]
            rowacc_a = acc_pool.tile([128, 4096], BF16)
            rowacc_b = acc_pool.tile([128, 4096], BF16)
            rowaccs = [rowacc_a, rowacc_b]

            # [128, 32/33] payload shapes parallelize the collective across
            # partitions (a [1, N] payload serializes on one partition row).
            # AllGather + a local 8-way fold is ~2x cheaper than AllReduce.
            ag1_in = dram_pool.tile([128, 32], BF16)
            ag1_out = dram_pool.tile([1024, 32], BF16, addr_space="Shared")
            ag2_in = dram_pool.tile([128, 33], BF16)
            ag2_out = dram_pool.tile([1024, 33], BF16, addr_space="Shared")
            bar_in = dram_pool.tile([1, 1], F32)
            bar_out = dram_pool.tile([8, 1], F32, addr_space="Shared")

            # dummy 4-byte AllGather: a barrier that absorbs cross-core
            # launch skew during the pipeline-fill head (where there is
            # idle slack) instead of at the first real collective
            nc.sync.dma_start(bar_in[:, :], ones_sb[0:1, 0:1])
            nc.gpsimd.collective_compute(
                "AllGather",
                mybir.AluOpType.bypass,
                replica_groups=[list(range(N_CORES))],
                ins=[bar_in[:, :].opt()],
                outs=[bar_out[:, :].opt()],
            )

            # ---- main loop over pairs of 512-column chunks ----
            for p in range(N_PAIR):
                if p == 4:
                    # cols 0..4095 complete: launch AllGather #1 now so it
                    # overlaps the rest of the loop (trigger emitted before
                    # pair 4's PAR so the gpsimd queue wait overlaps DVE)
                    nc.gpsimd.collective_compute(
                        "AllGather",
                        mybir.AluOpType.bypass,
                        replica_groups=[list(range(N_CORES))],
                        ins=[ag1_in[:, :].opt()],
                        outs=[ag1_out[:, :].opt()],
                    )
                pair_sb = chunk_pool.tile(
                    [128, 8192], BF16, name="pair_sb", tag="chunk"
                )
                # PE: S' tiles, 4 row-blocks at a time on the PE quadrants.
                # h = (g, cj): pair_sb layout [q, g:2, cj:2, r:4, f:512]
                for h in range(4):
                    g = h // 2
                    c = 2 * p + h % 2
                    mm_ps = mm_pool.tile(
                        [128, 4 * CHUNK], F32, name="mm_ps", tag="mm"
                    )
                    for r in range(4):
                        i = 4 * g + r
                        nc.tensor.matmul(
                            mm_ps[:, r * CHUNK:(r + 1) * CHUNK],
                            lhs_sb[32 * r:32 * r + K, i * 128:(i + 1) * 128],
                            rhs_sb[32 * r:32 * r + K, c * CHUNK:(c + 1) * CHUNK],
                            start=True,
                            stop=True,
                            tile_position=(32 * r, 0),
                        )
                    # ACT: PSUM fp32 -> SBUF bf16
                    nc.scalar.copy(
                        pair_sb[:, h * 2048:(h + 1) * 2048], mm_ps[:, :]
                    )

                pr = pair_sb.rearrange("q (g cj f) -> q g cj f", g=2, cj=2)
                if p == 0:
                    # row chain first and split per-g: the g=0 half starts
                    # after only two ACT copies (shrinks the pipeline head)
                    for g in range(2):
                        nc.vector.tensor_tensor(
                            rowacc_b[:, 2048 * g:2048 * (g + 1)],
                            pr[:, g, 0, :],
                            pr[:, g, 1, :],
                            MAX,
                        )

                # col tree (it feeds the PAR + collective chain):
                # contract g, then r-halves, on DVE.
                # t3[q, cj*512+f] = column m = 1024p + cj*512+f
                t1 = tree_pool.tile([128, 4096], BF16, name="t1", tag="t1")
                nc.vector.tensor_tensor(
                    t1[:, :], pair_sb[:, 0:4096], pair_sb[:, 4096:8192], MAX
                )
                t2 = tree_pool.tile([128, 2048], BF16, name="t2", tag="t2")
                t1v = t1.rearrange("q (cj r f) -> q cj r f", cj=2, r=4)
                t2v = t2.rearrange("q (cj r f) -> q cj r f", cj=2, r=2)
                nc.vector.tensor_tensor(
                    t2v[:, :, :, :], t1v[:, :, 0:2, :], t1v[:, :, 2:4, :], MAX
                )
                t3 = tree_pool.tile([128, 1024], BF16, name="t3", tag="t3")
                nc.vector.tensor_tensor(
                    t3.rearrange("q (cj f) -> q cj f", cj=2),
                    t2v[:, :, 0, :],
                    t2v[:, :, 1, :],
                    MAX,
                )
                # GpSimd partition fold: per-column max over all 128
                # partitions (result broadcast to every partition); dtype
                # matches the collective payload it feeds
                parx = tree_pool.tile([128, 1024], BF16, name="parx", tag="parx")
                nc.gpsimd.partition_all_reduce(
                    parx[:, :], t3[:, :], channels=128,
                    reduce_op=bass_isa.ReduceOp.max,
                )
                # stream partition 0 out to the collective payload:
                # pair p covers cols [8j+k] -> (partition j, col 8p'+k)
                if p < 4:
                    nc.sync.dma_start(
                        ag1_in[:, 8 * p:8 * (p + 1)], parx[0:1, :]
                    )
                else:
                    nc.sync.dma_start(
                        ag2_in[:, 8 * (p - 4):8 * (p - 3)], parx[0:1, :]
                    )

                # DVE row chain: max across the two chunks (cj), then into
                # rowacc.  view [q, g, cj, f'] with f' = r*512+f = 2048
                if p > 0:
                    trow = tree_pool.tile(
                        [128, 4096], BF16, name="trow", tag="trow"
                    )
                    nc.vector.tensor_tensor(
                        trow.rearrange("q (g f) -> q g f", g=2),
                        pr[:, :, 0, :],
                        pr[:, :, 1, :],
                        MAX,
                    )
                    src = rowaccs[p % 2]
                    dst = rowaccs[(p + 1) % 2]
                    nc.vector.tensor_tensor(dst[:, :], src[:, :], trow[:, :], MAX)

            # ---- local row-min finalization (negated: max chains) ----
            rowacc = rowaccs[N_PAIR % 2]
            rview = rowacc.rearrange("q (i f) -> q i f", i=8)
            rt1 = fin_pool.tile([128, 2048], BF16)
            rt2 = fin_pool.tile([128, 1024], BF16)
            r1v = rt1.rearrange("q (i f) -> q i f", i=8)
            r2v = rt2.rearrange("q (i f) -> q i f", i=8)
            nc.vector.tensor_tensor(
                r1v[:, :, :], rview[:, :, 0:256], rview[:, :, 256:512], MAX
            )
            nc.vector.tensor_tensor(
                r2v[:, :, :], r1v[:, :, 0:128], r1v[:, :, 128:256], MAX
            )
            # two more cheap 2x TT levels before the (1x-rate) reduce
            rt3 = fin_pool.tile([128, 512], BF16)
            rt4 = fin_pool.tile([128, 256], BF16)
            r3v = rt3.rearrange("q (i f) -> q i f", i=8)
            r4v = rt4.rearrange("q (i f) -> q i f", i=8)
            nc.vector.tensor_tensor(
                r3v[:, :, :], r2v[:, :, 0:64], r2v[:, :, 64:128], MAX
            )
            nc.vector.tensor_tensor(
                r4v[:, :, :], r3v[:, :, 0:32], r3v[:, :, 32:64], MAX
            )
            rowmax8 = fin_pool.tile([128, 8], F32)
            nc.vector.tensor_reduce(rowmax8[:, :], r4v, axis=X, op=MAX)

            # negate back to min-sq and clamp cancellation noise at 0:
            # rowmin8 = max(-rowmax8, 0), then sqrt and sum over free dim
            rowmin8 = fin_pool.tile([128, 8], F32)
            nc.vector.tensor_scalar(
                rowmin8[:, :], rowmax8[:, :], -1.0, 0.0, MULT, MAX
            )
            rowd = fin_pool.tile([128, 8], F32)
            rowpart = fin_pool.tile([128, 1], F32)
            nc.scalar.activation(
                rowd[:, :], rowmin8[:, :],
                mybir.ActivationFunctionType.Sqrt,
                accum_out=rowpart[:, :],
            )
            # partition sum -> scalar
            ps_row = mm_pool.tile([128, 4 * CHUNK], F32, name="ps_row", tag="mm")
            nc.tensor.matmul(
                ps_row[0:1, 0:1], ones_sb[:, :], rowpart[:, :],
                start=True, stop=True,
            )
            rowsum_sb = fin_pool.tile([1, 1], BF16)
            nc.scalar.copy(rowsum_sb[:, :], ps_row[0:1, 0:1])
            # this core's row-min sqrt-sum rides along in its own gathered
            # block at (row 0, col 32); other rows of col 32 are never read
            nc.scalar.dma_start(ag2_in[0:1, 32:33], rowsum_sb[:, :])

            # ---- AllGather #2 over [128, 33] fp32 ----
            nc.gpsimd.collective_compute(
                "AllGather",
                mybir.AluOpType.bypass,
                replica_groups=[list(range(N_CORES))],
                ins=[ag2_in[:, :].opt()],
                outs=[ag2_out[:, :].opt()],
            )

            # AG#1 consumption overlaps the AllGather #2 collective:
            # gather the 8 blocks (one strided DMA) and fold with a max tree
            cA8 = fin_pool.tile([128, 8 * 32], BF16)
            cAv = cA8.rearrange("q (j k) -> q j k", j=8)
            nc.scalar.dma_start(
                cAv[:, :, :],
                ag1_out[:, :].rearrange("(j q) k -> q j k", j=8),
            )
            mA1 = fin_pool.tile([128, 4 * 32], BF16)
            mA2 = fin_pool.tile([128, 2 * 32], BF16)
            cmaxA = fin_pool.tile([128, 32], BF16)
            mA1v = mA1.rearrange("q (j k) -> q j k", j=4)
            mA2v = mA2.rearrange("q (j k) -> q j k", j=2)
            nc.vector.tensor_tensor(
                mA1v[:, :, :], cAv[:, 0:4, :], cAv[:, 4:8, :], MAX
            )
            nc.vector.tensor_tensor(
                mA2v[:, :, :], mA1v[:, 0:2, :], mA1v[:, 2:4, :], MAX
            )
            nc.vector.tensor_tensor(
                cmaxA.rearrange("q (j k) -> q j k", j=1),
                mA2v[:, 0:1, :],
                mA2v[:, 1:2, :],
                MAX,
            )
            cminA = fin_pool.tile([128, 32], F32)
            nc.vector.tensor_scalar(
                cminA[:, :], cmaxA[:, :], -1.0, 0.0, MULT, MAX
            )
            cdA = fin_pool.tile([128, 32], F32)
            colpartA = fin_pool.tile([128, 1], F32)
            nc.scalar.activation(
                cdA[:, :], cminA[:, :],
                mybir.ActivationFunctionType.Sqrt,
                accum_out=colpartA[:, :],
            )

            # ---- global finalization (identical on every core) ----
            cB8 = fin_pool.tile([128, 8 * 33], BF16)
            cBv = cB8.rearrange("q (j k) -> q j k", j=8)
            nc.sync.dma_start(
                cBv[:, :, :],
                ag2_out[:, :].rearrange("(j q) k -> q j k", j=8),
            )
            mB1 = fin_pool.tile([128, 4 * 32], BF16)
            mB2 = fin_pool.tile([128, 2 * 32], BF16)
            cmaxB = fin_pool.tile([128, 32], BF16)
            mB1v = mB1.rearrange("q (j k) -> q j k", j=4)
            mB2v = mB2.rearrange("q (j k) -> q j k", j=2)
            nc.vector.tensor_tensor(
                mB1v[:, :, :], cBv[:, 0:4, 0:32], cBv[:, 4:8, 0:32], MAX
            )
            nc.vector.tensor_tensor(
                mB2v[:, :, :], mB1v[:, 0:2, :], mB1v[:, 2:4, :], MAX
            )
            nc.vector.tensor_tensor(
                cmaxB.rearrange("q (j k) -> q j k", j=1),
                mB2v[:, 0:1, :],
                mB2v[:, 1:2, :],
                MAX,
            )
            cminB = fin_pool.tile([128, 32], F32)
            nc.vector.tensor_scalar(
                cminB[:, :], cmaxB[:, :], -1.0, 0.0, MULT, MAX
            )
            cdB = fin_pool.tile([128, 32], F32)
            colpartB = fin_pool.tile([128, 1], F32)
            nc.scalar.activation(
                cdB[:, :], cminB[:, :],
                mybir.ActivationFunctionType.Sqrt,
                accum_out=colpartB[:, :],
            )
            colpart = fin_pool.tile([128, 1], F32)
            nc.vector.tensor_tensor(
                colpart[:, :], colpartA[:, :], colpartB[:, :], ADD
            )
            ps_col = mm_pool.tile([128, 4 * CHUNK], F32, name="ps_col", tag="mm")
            nc.tensor.matmul(
                ps_col[0:1, 0:1], ones_sb[:, :], colpart[:, :],
                start=True, stop=True,
            )
            colsum_sb = fin_pool.tile([1, 1], F32)
            nc.scalar.copy(colsum_sb[:, :], ps_col[0:1, 0:1])

            # all 8 cores' rowsums sit at (row 0, col 33j+32) of cB8
            rtot = fin_pool.tile([1, 1], F32)
            nc.vector.tensor_reduce(
                rtot[:, :], cBv[0:1, :, 32], axis=X, op=ADD
            )

            fin = fin_pool.tile([1, 1], F32)
            nc.vector.tensor_tensor(fin[:, :], colsum_sb[:, :], rtot[:, :], ADD)
            out_sb = fin_pool.tile([1, 1], F32)
            nc.scalar.mul(out_sb[:, :], fin[:, :], 1.0 / M)
            nc.sync.dma_start(out_d.ap(), out_sb[:, :])

    nc.compile()
    return nc


def _prep_inputs(y_pred, y_true):
    p = np.ascontiguousarray(np.asarray(y_pred, dtype=np.float32).reshape(-1, 2))
    t = np.ascontiguousarray(np.asarray(y_true, dtype=np.float32).reshape(-1, 2))
    assert p.shape == (N, 2) and t.shape == (M, 2)

    def split16(x):
        hi = x.astype(np.float16)
        lo = (x.astype(np.float64) - hi.astype(np.float64)).astype(np.float16)
        return hi, lo

    th, tl = split16(t)
    t2 = (t.astype(np.float64) ** 2).sum(1)
    t2h, t2l = split16(t2)
    ones_m = np.ones(M, np.float16)
    rhs10 = np.stack([
        th[:, 0], th[:, 1], th[:, 0], th[:, 1], t2h,
        ones_m, ones_m, tl[:, 0], tl[:, 1], t2l,
    ])
    rhs = np.ascontiguousarray(np.tile(rhs10, (4, 1)))

    in_maps = []
    for k in range(N_CORES):
        pk = p[k * N_LOC:(k + 1) * N_LOC]
        ph, pl = split16(pk)
        p2 = (pk.astype(np.float64) ** 2).sum(1)
        p2h, p2l = split16(p2)
        ones_n = np.ones(N_LOC, np.float16)
        # negated lhs: the PE emits S' = -(|p|^2 + |t|^2 - 2 p.t)
        lhs10 = np.stack([
            2.0 * ph[:, 0], 2.0 * ph[:, 1],
            2.0 * pl[:, 0], 2.0 * pl[:, 1],
            -ones_n, -p2h, -p2l,
            2.0 * ph[:, 0], 2.0 * ph[:, 1], -ones_n,
        ])
        lhs = np.ascontiguousarray(np.tile(lhs10, (4, 1)))
        in_maps.append({"lhs": lhs, "rhs": rhs})
    return in_maps


def kernel(y_pred, y_true):
    global LAST_RESULTS
    if "nc" not in _CACHE:
        _CACHE["nc"] = _build_program()
    nc = _CACHE["nc"]
    in_maps = _prep_inputs(y_pred, y_true)
    res = run_bass_kernel_spmd(
        nc,
        in_maps,
        core_ids=list(range(N_CORES)),
        trace=TRACE,
    )
    LAST_RESULTS = res
    return np.asarray(res.results[0]["out"], dtype=np.float32).reshape(())[()]
